# revision 59
# baseline (speedup 1.0000x reference)
"""MultiHeadAttention (B=2, S=2048, d_model=1024, 16 heads, causal) on 8 TRN2 cores.

Sharding: core i handles batch (i//4) and heads 4*(i%4) .. 4*(i%4)+4 (tensor
parallel over heads within a batch).  Each core computes its 4 heads'
Q/K/V projections, causal attention, and the partial output projection
(contribution of its 256 head-dims to all 1024 output dims).  The host sums
the 4 partials per batch and adds the output bias.

All matmuls run as float32r (TF32-like reduced precision, 1 cycle/row on the
PE at N>=256 vs 4 cycles/row for exact fp32).  The 1/sqrt(d_k) scale is
folded into Wq/bq on the host.

Layout notes (per core):
  xT   [1024, 2048]  x[b].T -> sbuf tiles [128, c(8), 512] per q-tile
  QT   [256, 2048]   per (d-chunk j, q-tile t): [128, 512]  (d on partitions)
  KT   same
  V    [2048, 4*65]  natural layout + ones col -> per k-chunk [128, 260]
  scoresT[k, q] = KT_h.T @ QT_h  (PE contracts over d=64; the two heads of a
       pair run in array row halves 0-63/64-127 via tile_position, so their
       K=64 matmuls can overlap on hardware)
  P = exp(scoresT)   (no max subtraction: |scores| <= ~3 so exp is safe;
       exp spans a chunk-pair [128, 2, 512] to amortize the ACT access bubble)
  causal mask: affine_select only on the two 128x128 diagonal triangle
       blocks; sub-diagonal garbage is excluded by PV matmul column ranges
  OT[d, q] (+ denom row) = [V_h | 1].T @ P      (ones col -> denominators)
  OT *= 1/denom  (DVE reciprocal + gpsimd partition_broadcast + DVE mult)
  out_partial[t, m] = OT.T @ WoT  (streamed per q-tile, DMA'd out)

Scheduling: one software-pipelined stage stream per (q-tile, head-pair);
projection groups for tile t+1 and output-projection groups for tile t-1
are queued and drained one per stage to fill PE slack while ACT (exp) paces
the stages.  All bulk DMAs are issued from the SP sequencer (each dma_start
costs ~1.3us of issuing-sequencer time and a full queue blocks the issuer).

Cost-model (TimelineSim) estimate: ~153 us/core; PE busy ~122 us (80%),
ACT ~87 us, DMA ~59 us.  Scale-relative absmax error vs the fp32 reference:
1.9e-4 (float32r input rounding).
"""

import numpy as np

import concourse.bass as bass
import concourse.tile as tile
import concourse.mybir as mybir
from concourse import bacc
from concourse.bass_utils import run_bass_kernel_spmd

dt = mybir.dt
AF = mybir.ActivationFunctionType

D_MODEL = 1024
N_HEADS = 16
D_K = 64
B = 2
S = 2048
H_PER_CORE = 4
DH = H_PER_CORE * D_K  # 256
N_CORES = 8
CCH = D_MODEL // 128  # 8 contraction chunks
QT_TILES = S // 512  # 4
KCH = S // 128  # 16 key chunks
VW = D_K + 1  # 65

_CACHE = {}

SC_BUFS = 2
MM_BUFS = 2
PV_BUFS = 1
PT_BUFS = 7


def _build():
    nc = bacc.Bacc("TRN2", target_bir_lowering=False, debug=False,
                   num_devices=N_CORES)

    xT = nc.dram_tensor("xT", [D_MODEL, S], dt.float32r, kind="ExternalInput").ap()
    WqT = nc.dram_tensor("WqT", [D_MODEL, DH], dt.float32r, kind="ExternalInput").ap()
    WkT = nc.dram_tensor("WkT", [D_MODEL, DH], dt.float32r, kind="ExternalInput").ap()
    WvT = nc.dram_tensor("WvT", [D_MODEL, DH], dt.float32r, kind="ExternalInput").ap()
    WoT = nc.dram_tensor("WoT", [DH, D_MODEL], dt.float32r, kind="ExternalInput").ap()
    bqs = nc.dram_tensor("bqs", [128, 2], dt.float32, kind="ExternalInput").ap()
    bks = nc.dram_tensor("bks", [128, 2], dt.float32, kind="ExternalInput").ap()
    bvb = nc.dram_tensor("bvb", [128, DH], dt.float32, kind="ExternalInput").ap()
    out = nc.dram_tensor("out", [S, D_MODEL], dt.float32, kind="ExternalOutput").ap()

    wq_d = WqT.rearrange("(c p) d -> c p d", p=128)
    wk_d = WkT.rearrange("(c p) d -> c p d", p=128)
    wv_d = WvT.rearrange("(c p) d -> c p d", p=128)

    with tile.TileContext(nc) as tc:
        with (
            tc.tile_pool(name="weights", bufs=1) as wpool,
            tc.tile_pool(name="acts", bufs=1) as apool,
            tc.tile_pool(name="pt", bufs=PT_BUFS) as ptpool,
            tc.tile_pool(name="sm", bufs=4) as smpool,
            tc.tile_pool(name="outsb", bufs=4) as opool,
            tc.tile_pool(name="mm", bufs=MM_BUFS, space="PSUM") as mmpool,
            tc.tile_pool(name="sc", bufs=SC_BUFS, space="PSUM") as scpool,
            tc.tile_pool(name="pv", bufs=1, space="PSUM") as pvpool,
        ):
            # ---- input loads: one big strided DMA per tensor / q-tile.
            # Each dma_start costs ~1.3us of issuing-sequencer time, so bulk
            # loads must be few and kept off the compute engines' sequencers
            # (everything goes through SP, ordered by first use).
            xT_t = [wpool.tile([128, CCH, 512], dt.float32r, name=f"xTt{t}",
                               tag=f"xTt{t}") for t in range(QT_TILES)]
            xT_r = xT.rearrange("(c p) q -> p c q", p=128)
            Wq_sb = wpool.tile([128, CCH, DH], dt.float32r)
            Wk_sb = wpool.tile([128, CCH, DH], dt.float32r)
            Wv_sb = wpool.tile([128, CCH, DH], dt.float32r)
            wq_r = WqT.rearrange("(c p) d -> p c d", p=128)
            # interleave chunk-halves so the first QK matmuls start early
            nc.sync.dma_start(Wq_sb[:, 0:4, :], wq_r[:, 0:4, :])
            nc.sync.dma_start(xT_t[0][:, 0:4, :], xT_r[:, 0:4, 0:512])
            nc.sync.dma_start(Wq_sb[:, 4:8, :], wq_r[:, 4:8, :])
            nc.sync.dma_start(xT_t[0][:, 4:8, :], xT_r[:, 4:8, 0:512])
            bqs_sb = wpool.tile([128, 2], dt.float32)
            nc.sync.dma_start(bqs_sb[:], bqs)
            nc.sync.dma_start(Wk_sb[:], WkT.rearrange("(c p) d -> p c d", p=128))
            bks_sb = wpool.tile([128, 2], dt.float32)
            nc.sync.dma_start(bks_sb[:], bks)
            nc.sync.dma_start(Wv_sb[:], WvT.rearrange("(c p) d -> p c d", p=128))
            bvb_sb = wpool.tile([128, DH], dt.float32)
            nc.sync.dma_start(bvb_sb[:], bvb)
            WoT_sb = [wpool.tile([128, D_MODEL], dt.float32r, name=f"Wo{j}",
                                 tag=f"Wo{j}") for j in range(2)]

            def emit_xt_load(t):
                # paced prefetch: issued one q-tile ahead
                nc.sync.dma_start(xT_t[t][:],
                                  xT_r[:, :, t * 512:(t + 1) * 512])

            # ---- fully streamed per q-tile: projections for tile t, then
            # attention for tile t (overlaps next tile's projections on PE),
            # then tile t's slice of the output projection.
            QT_sb = [[apool.tile([128, 512], dt.float32r, name=f"QT{j}_{t}",
                                 tag=f"QT{j}_{t}") for t in range(QT_TILES)]
                     for j in range(2)]
            KT_sb = [[apool.tile([128, 512], dt.float32r, name=f"KT{j}_{t}",
                                 tag=f"KT{j}_{t}") for t in range(QT_TILES)]
                     for j in range(2)]
            V_sb = [apool.tile([128, H_PER_CORE * VW], dt.float32r,
                               name=f"V{k}", tag=f"V{k}") for k in range(KCH)]
            OT_sb = [[apool.tile([128, 512], dt.float32r, name=f"OT{j}_{t}",
                                 tag=f"OT{j}_{t}") for t in range(QT_TILES)]
                     for j in range(2)]

            def emit_qk_proj(w, j, t):
                w_sb, o_sb, b_sb = ((Wq_sb, QT_sb, bqs_sb),
                                    (Wk_sb, KT_sb, bks_sb))[w]
                ps = mmpool.tile([128, 512], dt.float32, name="ps",
                                 tag="ps")
                for c in range(CCH):
                    nc.tensor.matmul(
                        ps[:],
                        w_sb[:, c, j * 128:(j + 1) * 128],
                        xT_t[t][:, c, :],
                        start=(c == 0), stop=(c == CCH - 1))
                nc.vector.tensor_scalar_add(
                    o_sb[j][t][:], ps[:], b_sb[:, j:j + 1])

            def emit_v_proj(k):
                ps = mmpool.tile([128, DH], dt.float32, name="ps", tag="ps")
                for c in range(CCH):
                    nc.tensor.matmul(
                        ps[:],
                        xT_t[k // 4][:, c, (k % 4) * 128:(k % 4 + 1) * 128],
                        Wv_sb[:, c, :],
                        start=(c == 0), stop=(c == CCH - 1))
                v_dst = V_sb[k][:].rearrange("p (h e) -> p h e", e=VW)[:, :, 0:D_K]
                nc.vector.tensor_tensor(
                    out=v_dst,
                    in0=ps[:].rearrange("p (h e) -> p h e", e=D_K),
                    in1=bvb_sb[:].rearrange("p (h e) -> p h e", e=D_K),
                    op=mybir.AluOpType.add)
                od = V_sb[k][:].rearrange("p (h e) -> p h e", e=VW)[:, :, D_K]
                nc.vector.tensor_scalar(
                    out=od, in0=bvb_sb[:, 0:H_PER_CORE], scalar1=0.0,
                    scalar2=1.0, op0=mybir.AluOpType.mult,
                    op1=mybir.AluOpType.add)

            # work queues drained one item per attention pair-stage: proj
            # groups for the next q-tile first (they gate the next tile's
            # attention), then output-projection groups (no dependents).
            pending_proj = []
            pending_oproj = []

            def drain_one():
                if pending_proj:
                    pending_proj.pop(0)()
                elif pending_oproj:
                    emit_oproj_group(*pending_oproj.pop(0))

            def emit_oproj_group(t, tt, on_act=False):
                o_sb = opool.tile([128, 2, 512], dt.float32, name="osb",
                                  tag="osb", bufs=2)
                for m in range(2):
                    ps = mmpool.tile([128, 512], dt.float32, name="ps",
                                     tag="ps")
                    for j in range(2):
                        nc.tensor.matmul(
                            ps[:],
                            OT_sb[j][t][:, (tt % 4) * 128:(tt % 4 + 1) * 128],
                            WoT_sb[j][:, m * 512:(m + 1) * 512],
                            start=(j == 0), stop=(j == 1))
                    if on_act:
                        nc.scalar.copy(o_sb[:, m, :], ps[:])
                    else:
                        nc.vector.tensor_copy(o_sb[:, m, :], ps[:])
                nc.sync.dma_start(out[tt * 128:(tt + 1) * 128, :], o_sb[:])

            from functools import partial

            def queue_proj(t):
                pending_proj.append(partial(emit_qk_proj, 0, 0, t))
                pending_proj.append(partial(emit_qk_proj, 1, 0, t))
                for k in range(4 * t, 4 * t + 4):
                    pending_proj.append(partial(emit_v_proj, k))
                pending_proj.append(partial(emit_qk_proj, 0, 1, t))
                pending_proj.append(partial(emit_qk_proj, 1, 1, t))

            queue_proj(0)
            for t in range(QT_TILES):
                npair = 2 * (t + 1)
                if t + 1 < QT_TILES:
                    emit_xt_load(t + 1)
                if t == 0:
                    for j in range(2):
                        nc.sync.dma_start(WoT_sb[j][:],
                                          WoT[j * 128:(j + 1) * 128, :])
                # flush any proj groups for this tile not yet drained
                while pending_proj:
                    pending_proj.pop(0)()
                if t + 1 < QT_TILES:
                    queue_proj(t + 1)
                for j in range(2):  # head pair (heads 2j, 2j+1)
                    pvs = [pvpool.tile([VW, 512], dt.float32, name=f"pv{p}",
                                       tag=f"pv{p}", bufs=PV_BUFS) for p in range(2)]
                    # Diagonal pairs first: pair 2t covers all pv columns with
                    # its two start=True matmuls; later pairs accumulate.
                    # Sub-diagonal exp garbage is excluded by restricting the
                    # PV matmul column ranges; only the two 128x128 triangle
                    # blocks get masked (single strided affine_select).
                    # Software-pipelined by one pair-stage: PV matmuls of pair
                    # i-1 are emitted after pair i's scores+exp, so the PE
                    # never head-of-line blocks on an exp in flight.
                    def flush(mms_pp, final):
                        for p in range(2):
                            for i, (lhsT, rhs, o, st) in enumerate(mms_pp[p]):
                                nc.tensor.matmul(
                                    o, lhsT, rhs, start=st,
                                    stop=(final and i == len(mms_pp[p]) - 1),
                                    skip_group_check=True)

                    prev = None
                    for pii, pi in enumerate(range(npair)):
                        diag = pi >= 2 * t
                        first_pair = pii == 0
                        pts = []
                        for p in range(2):
                            lo = p * 64
                            sc = scpool.tile([128, 2, 512], dt.float32,
                                             name="sc", tag="sc")
                            for half in range(2):
                                c = 2 * pi + half
                                nc.tensor.matmul(
                                    sc[:, half, :],
                                    KT_sb[j][(2 * pi) // 4][
                                        lo:lo + 64,
                                        (c % 4) * 128:(c % 4 + 1) * 128],
                                    QT_sb[j][t][lo:lo + 64, :],
                                    start=True, stop=True,
                                    tile_position=(lo, 0))
                            pt = ptpool.tile([128, 2, 512], dt.float32r,
                                             name="pt", tag="pt")
                            nc.scalar.activation(pt[:], sc[:], AF.Exp)
                            pts.append(pt)
                        mms_pp = []
                        for p in range(2):
                            pt = pts[p]
                            h = 2 * j + p
                            vsl = lambda c: V_sb[c][:, h * VW:(h + 1) * VW]
                            mms = []  # (lhsT, rhs, out, start)
                            if not diag:
                                for half in range(2):
                                    mms.append((vsl(2 * pi + half),
                                                pt[:, half, :], pvs[p][:],
                                                first_pair and half == 0))
                            else:
                                r0 = 2 * (pi - 2 * t)
                                # zero below-diagonal of each 128x128
                                # triangle block
                                for half in range(2):
                                    r = r0 + half
                                    tri = pt[:, half, 128 * r:128 * (r + 1)]
                                    nc.gpsimd.affine_select(
                                        out=tri, in_=tri,
                                        compare_op=mybir.AluOpType.is_ge,
                                        fill=0.0, base=0,
                                        pattern=[[1, 128]],
                                        channel_multiplier=-1)
                                for half in range(2):
                                    # one matmul over [128r, 512): triangle
                                    # (masked) + fully-valid columns
                                    r = r0 + half
                                    mms.append(
                                        (vsl(2 * pi + half),
                                         pt[:, half, 128 * r:],
                                         pvs[p][:, 128 * r:],
                                         first_pair and half == 0))
                            mms_pp.append(mms)
                        if prev is not None:
                            drain_one()
                            flush(prev, False)
                        prev = mms_pp
                    flush(prev, True)
                    for p in range(2):
                        rc = smpool.tile([1, 512], dt.float32, name="rc",
                                         tag="rc", bufs=2)
                        nc.vector.reciprocal(rc[:], pvs[p][D_K:VW, :])
                        bc = smpool.tile([64, 512], dt.float32, name="bc",
                                         tag="bc", bufs=2)
                        nc.gpsimd.partition_broadcast(bc[:], rc[:])
                        nc.vector.tensor_tensor(
                            out=OT_sb[j][t][p * 64:(p + 1) * 64, :],
                            in0=pvs[p][0:D_K, :], in1=bc[:],
                            op=mybir.AluOpType.mult)


                # queue this q-tile's output projection; drained one group
                # per attention pair-stage of later tiles (fills PE slack
                # while ACT is the stage bottleneck)
                for tt in range(4 * t, 4 * t + 4):
                    pending_oproj.append((t, tt))
            for i, args in enumerate(pending_oproj):
                emit_oproj_group(*args, on_act=(i % 2 == 0))
    nc.compile()
    return nc


def _in_maps(x, Wq, bq, Wk, bk, Wv, bv, Wo, bo):
    maps = []
    for core in range(N_CORES):
        b = core // 4
        h0 = (core % 4) * H_PER_CORE
        hs = slice(h0 * D_K, (h0 + H_PER_CORE) * D_K)
        m = {
            "xT": np.ascontiguousarray(x[b].T),
            "WqT": np.ascontiguousarray(Wq[hs, :].T) * 0.125,
            "WkT": np.ascontiguousarray(Wk[hs, :].T),
            "WvT": np.ascontiguousarray(Wv[hs, :].T),
            "WoT": np.ascontiguousarray(Wo[:, hs].T),
            "bqs": np.ascontiguousarray((bq[hs] * 0.125).reshape(2, 128).T),
            "bks": np.ascontiguousarray(bk[hs].reshape(2, 128).T),
            "bvb": np.ascontiguousarray(
                np.broadcast_to(bv[hs], (128, DH))),
        }
        maps.append({k: np.ascontiguousarray(v, dtype=np.float32)
                     for k, v in m.items()})
    return maps


def kernel(x, Wq, bq, Wk, bk, Wv, bv, Wo, bo, _trace=False):
    if "nc" not in _CACHE:
        _CACHE["nc"] = _build()
    nc = _CACHE["nc"]
    in_maps = _in_maps(np.asarray(x, dtype=np.float32),
                       np.asarray(Wq, dtype=np.float32),
                       np.asarray(bq, dtype=np.float32),
                       np.asarray(Wk, dtype=np.float32),
                       np.asarray(bk, dtype=np.float32),
                       np.asarray(Wv, dtype=np.float32),
                       np.asarray(bv, dtype=np.float32),
                       np.asarray(Wo, dtype=np.float32),
                       np.asarray(bo, dtype=np.float32))
    res = run_bass_kernel_spmd(nc, in_maps, core_ids=list(range(N_CORES)),
                               trace=_trace)
    bo = np.asarray(bo, dtype=np.float32)
    out = np.zeros((B, S, D_MODEL), dtype=np.float32)
    for b in range(B):
        acc = res.results[b * 4]["out"].astype(np.float64)
        for core in range(b * 4 + 1, b * 4 + 4):
            acc = acc + res.results[core]["out"]
        out[b] = (acc + bo).astype(np.float32)
    if _trace:
        return out, res
    return out


# revision 60
# speedup vs baseline: 1.0279x; 1.0279x over previous
"""MultiHeadAttention (B=2, S=2048, d_model=1024, 16 heads, causal) on 8 TRN2 cores.

Sharding: core i handles batch (i//4) and heads 4*(i%4) .. 4*(i%4)+4 (tensor
parallel over heads within a batch).  Each core computes its 4 heads'
Q/K/V projections, causal attention, and the partial output projection
(contribution of its 256 head-dims to all 1024 output dims).  The host sums
the 4 partials per batch and adds the output bias.

All matmuls run as float32r (TF32-like reduced precision, 1 cycle/row on the
PE at N>=256 vs 4 cycles/row for exact fp32).  The 1/sqrt(d_k) scale is
folded into Wq/bq on the host.

Layout notes (per core):
  xT   [1024, 2048]  x[b].T -> sbuf tiles [128, c(8), 512] per q-tile
  QT   [256, 2048]   per (d-chunk j, q-tile t): [128, 512]  (d on partitions)
  KT   same
  V    [2048, 4*65]  natural layout + ones col -> per k-chunk [128, 260]
  scoresT[k, q] = KT_h.T @ QT_h  (PE contracts over d=64; the two heads of a
       pair run in array row halves 0-63/64-127 via tile_position, so their
       K=64 matmuls can overlap on hardware)
  P = exp(scoresT)   (no max subtraction: |scores| <= ~3 so exp is safe;
       exp spans a chunk-pair [128, 2, 512] to amortize the ACT access bubble)
  causal mask: affine_select only on the two 128x128 diagonal triangle
       blocks; sub-diagonal garbage is excluded by PV matmul column ranges
  OT[d, q] (+ denom row) = [V_h | 1].T @ P      (ones col -> denominators)
  OT *= 1/denom  (DVE reciprocal + gpsimd partition_broadcast + DVE mult)
  out_partial[t, m] = OT.T @ WoT  (streamed per q-tile, DMA'd out)

Scheduling: one software-pipelined stage stream per (q-tile, head-pair);
projection groups for tile t+1 and output-projection groups for tile t-1
are queued and drained one per stage to fill PE slack while ACT (exp) paces
the stages.  All bulk DMAs are issued from the SP sequencer (each dma_start
costs ~1.3us of issuing-sequencer time and a full queue blocks the issuer).

Cost-model (TimelineSim) estimate: ~153 us/core; PE busy ~122 us (80%),
ACT ~87 us, DMA ~59 us.  Scale-relative absmax error vs the fp32 reference:
1.9e-4 (float32r input rounding).
"""

import numpy as np

import concourse.bass as bass
import concourse.tile as tile
import concourse.mybir as mybir
from concourse import bacc
from concourse.bass_utils import run_bass_kernel_spmd

dt = mybir.dt
AF = mybir.ActivationFunctionType

D_MODEL = 1024
N_HEADS = 16
D_K = 64
B = 2
S = 2048
H_PER_CORE = 4
DH = H_PER_CORE * D_K  # 256
N_CORES = 8
CCH = D_MODEL // 128  # 8 contraction chunks
QT_TILES = S // 512  # 4
KCH = S // 128  # 16 key chunks
VW = D_K + 1  # 65

_CACHE = {}

SC_BUFS = 2
MM_BUFS = 2
PV_BUFS = 1
PT_BUFS = 7


def _build():
    nc = bacc.Bacc("TRN2", target_bir_lowering=False, debug=False,
                   num_devices=N_CORES)

    xT = nc.dram_tensor("xT", [D_MODEL, S], dt.float32r, kind="ExternalInput").ap()
    WqT = nc.dram_tensor("WqT", [D_MODEL, DH], dt.float32r, kind="ExternalInput").ap()
    WkT = nc.dram_tensor("WkT", [D_MODEL, DH], dt.float32r, kind="ExternalInput").ap()
    WvT = nc.dram_tensor("WvT", [D_MODEL, DH], dt.float32r, kind="ExternalInput").ap()
    WoT = nc.dram_tensor("WoT", [DH, D_MODEL], dt.float32r, kind="ExternalInput").ap()
    bqs = nc.dram_tensor("bqs", [128, 2], dt.float32, kind="ExternalInput").ap()
    bks = nc.dram_tensor("bks", [128, 2], dt.float32, kind="ExternalInput").ap()
    bvb = nc.dram_tensor("bvb", [128, DH], dt.float32, kind="ExternalInput").ap()
    out = nc.dram_tensor("out", [S, D_MODEL], dt.float32, kind="ExternalOutput").ap()

    wq_d = WqT.rearrange("(c p) d -> c p d", p=128)
    wk_d = WkT.rearrange("(c p) d -> c p d", p=128)
    wv_d = WvT.rearrange("(c p) d -> c p d", p=128)

    with tile.TileContext(nc) as tc:
        with (
            tc.tile_pool(name="weights", bufs=1) as wpool,
            tc.tile_pool(name="acts", bufs=1) as apool,
            tc.tile_pool(name="pt", bufs=PT_BUFS) as ptpool,
            tc.tile_pool(name="sm", bufs=4) as smpool,
            tc.tile_pool(name="outsb", bufs=4) as opool,
            tc.tile_pool(name="mm", bufs=MM_BUFS, space="PSUM") as mmpool,
            tc.tile_pool(name="sc", bufs=SC_BUFS, space="PSUM") as scpool,
            tc.tile_pool(name="pv", bufs=1, space="PSUM") as pvpool,
        ):
            # ---- input loads: one big strided DMA per tensor / q-tile.
            # Each dma_start costs ~1.3us of issuing-sequencer time, so bulk
            # loads must be few and kept off the compute engines' sequencers
            # (everything goes through SP, ordered by first use).
            xT_t = [wpool.tile([128, CCH, 512], dt.float32r, name=f"xTt{t}",
                               tag=f"xTt{t}") for t in range(QT_TILES)]
            xT_r = xT.rearrange("(c p) q -> p c q", p=128)
            Wq_sb = wpool.tile([128, CCH, DH], dt.float32r)
            Wk_sb = wpool.tile([128, CCH, DH], dt.float32r)
            Wv_sb = wpool.tile([128, CCH, DH], dt.float32r)
            wq_r = WqT.rearrange("(c p) d -> p c d", p=128)
            # interleave chunk-halves so the first QK matmuls start early
            nc.sync.dma_start(Wq_sb[:, 0:4, :], wq_r[:, 0:4, :])
            nc.sync.dma_start(xT_t[0][:, 0:4, :], xT_r[:, 0:4, 0:512])
            nc.sync.dma_start(Wq_sb[:, 4:8, :], wq_r[:, 4:8, :])
            nc.sync.dma_start(xT_t[0][:, 4:8, :], xT_r[:, 4:8, 0:512])
            bqs_sb = wpool.tile([128, 2], dt.float32)
            nc.sync.dma_start(bqs_sb[:], bqs)
            nc.sync.dma_start(Wk_sb[:], WkT.rearrange("(c p) d -> p c d", p=128))
            bks_sb = wpool.tile([128, 2], dt.float32)
            nc.sync.dma_start(bks_sb[:], bks)
            nc.sync.dma_start(Wv_sb[:], WvT.rearrange("(c p) d -> p c d", p=128))
            bvb_sb = wpool.tile([128, DH], dt.float32)
            nc.sync.dma_start(bvb_sb[:], bvb)
            WoT_sb = [wpool.tile([128, D_MODEL], dt.float32r, name=f"Wo{j}",
                                 tag=f"Wo{j}") for j in range(2)]

            def emit_xt_load(t):
                # paced prefetch: issued one q-tile ahead
                nc.sync.dma_start(xT_t[t][:],
                                  xT_r[:, :, t * 512:(t + 1) * 512])

            # ---- fully streamed per q-tile: projections for tile t, then
            # attention for tile t (overlaps next tile's projections on PE),
            # then tile t's slice of the output projection.
            QT_sb = [[apool.tile([128, 512], dt.float32r, name=f"QT{j}_{t}",
                                 tag=f"QT{j}_{t}") for t in range(QT_TILES)]
                     for j in range(2)]
            KT_sb = [[apool.tile([128, 512], dt.float32r, name=f"KT{j}_{t}",
                                 tag=f"KT{j}_{t}") for t in range(QT_TILES)]
                     for j in range(2)]
            V_sb = [apool.tile([128, H_PER_CORE * VW], dt.float32r,
                               name=f"V{k}", tag=f"V{k}") for k in range(KCH)]
            OT_sb = [[apool.tile([128, 512], dt.float32r, name=f"OT{j}_{t}",
                                 tag=f"OT{j}_{t}") for t in range(QT_TILES)]
                     for j in range(2)]

            def emit_qk_proj(w, j, t):
                w_sb, o_sb, b_sb = ((Wq_sb, QT_sb, bqs_sb),
                                    (Wk_sb, KT_sb, bks_sb))[w]
                ps = mmpool.tile([128, 512], dt.float32, name="ps",
                                 tag="ps")
                for c in range(CCH):
                    nc.tensor.matmul(
                        ps[:],
                        w_sb[:, c, j * 128:(j + 1) * 128],
                        xT_t[t][:, c, :],
                        start=(c == 0), stop=(c == CCH - 1))
                nc.vector.tensor_scalar_add(
                    o_sb[j][t][:], ps[:], b_sb[:, j:j + 1])

            def emit_v_proj(k):
                ps = mmpool.tile([128, DH], dt.float32, name="ps", tag="ps")
                for c in range(CCH):
                    nc.tensor.matmul(
                        ps[:],
                        xT_t[k // 4][:, c, (k % 4) * 128:(k % 4 + 1) * 128],
                        Wv_sb[:, c, :],
                        start=(c == 0), stop=(c == CCH - 1))
                v_dst = V_sb[k][:].rearrange("p (h e) -> p h e", e=VW)[:, :, 0:D_K]
                nc.vector.tensor_tensor(
                    out=v_dst,
                    in0=ps[:].rearrange("p (h e) -> p h e", e=D_K),
                    in1=bvb_sb[:].rearrange("p (h e) -> p h e", e=D_K),
                    op=mybir.AluOpType.add)
                od = V_sb[k][:].rearrange("p (h e) -> p h e", e=VW)[:, :, D_K]
                nc.vector.tensor_scalar(
                    out=od, in0=bvb_sb[:, 0:H_PER_CORE], scalar1=0.0,
                    scalar2=1.0, op0=mybir.AluOpType.mult,
                    op1=mybir.AluOpType.add)

            # work queues drained one item per attention pair-stage: proj
            # groups for the next q-tile first (they gate the next tile's
            # attention), then output-projection groups (no dependents).
            pending_proj = []
            pending_oproj = []

            def drain_one():
                if pending_proj:
                    pending_proj.pop(0)()
                elif pending_oproj:
                    emit_oproj_group(*pending_oproj.pop(0))

            def emit_oproj_group(t, tt, on_act=False):
                o_sb = opool.tile([128, 2, 512], dt.float32, name="osb",
                                  tag="osb", bufs=2)
                for m in range(2):
                    ps = mmpool.tile([128, 512], dt.float32, name="ps",
                                     tag="ps")
                    for j in range(2):
                        nc.tensor.matmul(
                            ps[:],
                            OT_sb[j][t][:, (tt % 4) * 128:(tt % 4 + 1) * 128],
                            WoT_sb[j][:, m * 512:(m + 1) * 512],
                            start=(j == 0), stop=(j == 1))
                    if on_act:
                        nc.scalar.copy(o_sb[:, m, :], ps[:])
                    else:
                        nc.vector.tensor_copy(o_sb[:, m, :], ps[:])
                nc.sync.dma_start(out[tt * 128:(tt + 1) * 128, :], o_sb[:])

            from functools import partial

            def queue_proj(t):
                pending_proj.append(partial(emit_qk_proj, 0, 0, t))
                pending_proj.append(partial(emit_qk_proj, 1, 0, t))
                for k in range(4 * t, 4 * t + 4):
                    pending_proj.append(partial(emit_v_proj, k))
                pending_proj.append(partial(emit_qk_proj, 0, 1, t))
                pending_proj.append(partial(emit_qk_proj, 1, 1, t))

            queue_proj(0)
            for t in range(QT_TILES):
                npair = 2 * (t + 1)
                if t + 1 < QT_TILES:
                    emit_xt_load(t + 1)
                if t == 0:
                    for j in range(2):
                        nc.sync.dma_start(WoT_sb[j][:],
                                          WoT[j * 128:(j + 1) * 128, :])
                # flush any proj groups for this tile not yet drained
                while pending_proj:
                    pending_proj.pop(0)()
                if t + 1 < QT_TILES:
                    queue_proj(t + 1)
                for j in range(2):  # head pair (heads 2j, 2j+1)
                    pvs = [pvpool.tile([VW, 512], dt.float32, name=f"pv{p}",
                                       tag=f"pv{p}", bufs=PV_BUFS) for p in range(2)]
                    # Diagonal pairs first: pair 2t covers all pv columns with
                    # its two start=True matmuls; later pairs accumulate.
                    # Sub-diagonal exp garbage is excluded by restricting the
                    # PV matmul column ranges; only the two 128x128 triangle
                    # blocks get masked (single strided affine_select).
                    # Software-pipelined by one pair-stage: PV matmuls of pair
                    # i-1 are emitted after pair i's scores+exp, so the PE
                    # never head-of-line blocks on an exp in flight.
                    def flush(mms_pp, final):
                        for p in range(2):
                            for i, (lhsT, rhs, o, st) in enumerate(mms_pp[p]):
                                nc.tensor.matmul(
                                    o, lhsT, rhs, start=st,
                                    stop=(final and i == len(mms_pp[p]) - 1),
                                    skip_group_check=True)

                    prev = None
                    for pii, pi in enumerate(range(npair)):
                        diag = pi >= 2 * t
                        first_pair = pii == 0
                        # for the second diagonal pair (r0=2) only columns
                        # q >= 128*r0 can be unmasked for either half, so the
                        # scores matmuls and exp skip the dead columns.
                        q0 = 128 * 2 * (pi - 2 * t) if diag else 0
                        pts = []
                        for p in range(2):
                            lo = p * 64
                            sc = scpool.tile([128, 2, 512], dt.float32,
                                             name="sc", tag="sc")
                            for half in range(2):
                                c = 2 * pi + half
                                nc.tensor.matmul(
                                    sc[:, half, q0:],
                                    KT_sb[j][(2 * pi) // 4][
                                        lo:lo + 64,
                                        (c % 4) * 128:(c % 4 + 1) * 128],
                                    QT_sb[j][t][lo:lo + 64, q0:],
                                    start=True, stop=True,
                                    tile_position=(lo, 0))
                            pt = ptpool.tile([128, 2, 512], dt.float32r,
                                             name="pt", tag="pt")
                            nc.scalar.activation(pt[:, :, q0:], sc[:, :, q0:],
                                                 AF.Exp)
                            pts.append(pt)
                        mms_pp = []
                        for p in range(2):
                            pt = pts[p]
                            h = 2 * j + p
                            vsl = lambda c: V_sb[c][:, h * VW:(h + 1) * VW]
                            mms = []  # (lhsT, rhs, out, start)
                            if not diag:
                                for half in range(2):
                                    mms.append((vsl(2 * pi + half),
                                                pt[:, half, :], pvs[p][:],
                                                first_pair and half == 0))
                            else:
                                r0 = 2 * (pi - 2 * t)
                                # zero below-diagonal of each 128x128
                                # triangle block
                                for half in range(2):
                                    r = r0 + half
                                    tri = pt[:, half, 128 * r:128 * (r + 1)]
                                    nc.gpsimd.affine_select(
                                        out=tri, in_=tri,
                                        compare_op=mybir.AluOpType.is_ge,
                                        fill=0.0, base=0,
                                        pattern=[[1, 128]],
                                        channel_multiplier=-1)
                                for half in range(2):
                                    # one matmul over [128r, 512): triangle
                                    # (masked) + fully-valid columns
                                    r = r0 + half
                                    mms.append(
                                        (vsl(2 * pi + half),
                                         pt[:, half, 128 * r:],
                                         pvs[p][:, 128 * r:],
                                         first_pair and half == 0))
                            mms_pp.append(mms)
                        if prev is not None:
                            drain_one()
                            flush(prev, False)
                        prev = mms_pp
                    flush(prev, True)
                    for p in range(2):
                        rc = smpool.tile([1, 512], dt.float32, name="rc",
                                         tag="rc", bufs=2)
                        nc.vector.reciprocal(rc[:], pvs[p][D_K:VW, :])
                        bc = smpool.tile([64, 512], dt.float32, name="bc",
                                         tag="bc", bufs=2)
                        nc.gpsimd.partition_broadcast(bc[:], rc[:])
                        nc.vector.tensor_tensor(
                            out=OT_sb[j][t][p * 64:(p + 1) * 64, :],
                            in0=pvs[p][0:D_K, :], in1=bc[:],
                            op=mybir.AluOpType.mult)


                # queue this q-tile's output projection; drained one group
                # per attention pair-stage of later tiles (fills PE slack
                # while ACT is the stage bottleneck)
                for tt in range(4 * t, 4 * t + 4):
                    pending_oproj.append((t, tt))
            for i, args in enumerate(pending_oproj):
                emit_oproj_group(*args, on_act=(i % 2 == 0))
    nc.compile()
    return nc


def _in_maps(x, Wq, bq, Wk, bk, Wv, bv, Wo, bo):
    maps = []
    for core in range(N_CORES):
        b = core // 4
        h0 = (core % 4) * H_PER_CORE
        hs = slice(h0 * D_K, (h0 + H_PER_CORE) * D_K)
        m = {
            "xT": np.ascontiguousarray(x[b].T),
            "WqT": np.ascontiguousarray(Wq[hs, :].T) * 0.125,
            "WkT": np.ascontiguousarray(Wk[hs, :].T),
            "WvT": np.ascontiguousarray(Wv[hs, :].T),
            "WoT": np.ascontiguousarray(Wo[:, hs].T),
            "bqs": np.ascontiguousarray((bq[hs] * 0.125).reshape(2, 128).T),
            "bks": np.ascontiguousarray(bk[hs].reshape(2, 128).T),
            "bvb": np.ascontiguousarray(
                np.broadcast_to(bv[hs], (128, DH))),
        }
        maps.append({k: np.ascontiguousarray(v, dtype=np.float32)
                     for k, v in m.items()})
    return maps


def kernel(x, Wq, bq, Wk, bk, Wv, bv, Wo, bo, _trace=False):
    if "nc" not in _CACHE:
        _CACHE["nc"] = _build()
    nc = _CACHE["nc"]
    in_maps = _in_maps(np.asarray(x, dtype=np.float32),
                       np.asarray(Wq, dtype=np.float32),
                       np.asarray(bq, dtype=np.float32),
                       np.asarray(Wk, dtype=np.float32),
                       np.asarray(bk, dtype=np.float32),
                       np.asarray(Wv, dtype=np.float32),
                       np.asarray(bv, dtype=np.float32),
                       np.asarray(Wo, dtype=np.float32),
                       np.asarray(bo, dtype=np.float32))
    res = run_bass_kernel_spmd(nc, in_maps, core_ids=list(range(N_CORES)),
                               trace=_trace)
    bo = np.asarray(bo, dtype=np.float32)
    out = np.zeros((B, S, D_MODEL), dtype=np.float32)
    for b in range(B):
        acc = res.results[b * 4]["out"].astype(np.float64)
        for core in range(b * 4 + 1, b * 4 + 4):
            acc = acc + res.results[core]["out"]
        out[b] = (acc + bo).astype(np.float32)
    if _trace:
        return out, res
    return out


# revision 65
# speedup vs baseline: 1.0361x; 1.0080x over previous
"""MultiHeadAttention (B=2, S=2048, d_model=1024, 16 heads, causal) on 8 TRN2 cores.

Sharding: core i handles batch (i//4) and heads 4*(i%4) .. 4*(i%4)+4 (tensor
parallel over heads within a batch).  Each core computes its 4 heads'
Q/K/V projections, causal attention, and the partial output projection
(contribution of its 256 head-dims to all 1024 output dims).  The host sums
the 4 partials per batch and adds the output bias.

All matmuls run as float32r (TF32-like reduced precision, 1 cycle/row on the
PE at N>=256 vs 4 cycles/row for exact fp32).  The 1/sqrt(d_k) scale is
folded into Wq/bq on the host.

Layout notes (per core):
  xT   [1024, 2048]  x[b].T -> sbuf tiles [128, c(8), 512] per q-tile
  QT   [256, 2048]   per (d-chunk j, q-tile t): [128, 512]  (d on partitions)
  KT   same
  V    [2048, 4*65]  natural layout + ones col -> per k-chunk [128, 260]
  scoresT[k, q] = KT_h.T @ QT_h  (PE contracts over d=64; the two heads of a
       pair run in array row halves 0-63/64-127 via tile_position, so their
       K=64 matmuls can overlap on hardware)
  P = exp(scoresT)   (no max subtraction: |scores| <= ~3 so exp is safe;
       exp spans a chunk-pair [128, 2, 512] to amortize the ACT access bubble)
  causal mask: affine_select only on the two 128x128 diagonal triangle
       blocks; sub-diagonal garbage is excluded by PV matmul column ranges
  OT[d, q] (+ denom row) = [V_h | 1].T @ P      (ones col -> denominators)
  OT *= 1/denom  (DVE reciprocal + gpsimd partition_broadcast + DVE mult)
  out_partial[t, m] = OT.T @ WoT  (streamed per q-tile, DMA'd out)

Scheduling: one software-pipelined stage stream per (q-tile, head-pair);
projection groups for tile t+1 and output-projection groups for tile t-1
are queued and drained one per stage to fill PE slack while ACT (exp) paces
the stages.  All bulk DMAs are issued from the SP sequencer (each dma_start
costs ~1.3us of issuing-sequencer time and a full queue blocks the issuer).

Cost-model (TimelineSim) estimate: ~153 us/core; PE busy ~122 us (80%),
ACT ~87 us, DMA ~59 us.  Scale-relative absmax error vs the fp32 reference:
1.9e-4 (float32r input rounding).
"""

import numpy as np

import concourse.bass as bass
import concourse.tile as tile
import concourse.mybir as mybir
from concourse import bacc
from concourse.bass_utils import run_bass_kernel_spmd

dt = mybir.dt
AF = mybir.ActivationFunctionType

D_MODEL = 1024
N_HEADS = 16
D_K = 64
B = 2
S = 2048
H_PER_CORE = 4
DH = H_PER_CORE * D_K  # 256
N_CORES = 8
CCH = D_MODEL // 128  # 8 contraction chunks
QT_TILES = S // 512  # 4
KCH = S // 128  # 16 key chunks
VW = D_K + 1  # 65

_CACHE = {}

SC_BUFS = 2
MM_BUFS = 2
PV_BUFS = 1
PT_BUFS = 7


def _build():
    nc = bacc.Bacc("TRN2", target_bir_lowering=False, debug=False,
                   num_devices=N_CORES)

    xT = nc.dram_tensor("xT", [D_MODEL, S], dt.float32r, kind="ExternalInput").ap()
    WqT = nc.dram_tensor("WqT", [D_MODEL, DH], dt.float32r, kind="ExternalInput").ap()
    WkT = nc.dram_tensor("WkT", [D_MODEL, DH], dt.float32r, kind="ExternalInput").ap()
    WvT = nc.dram_tensor("WvT", [D_MODEL, DH], dt.float32r, kind="ExternalInput").ap()
    WoT = nc.dram_tensor("WoT", [DH, D_MODEL], dt.float32r, kind="ExternalInput").ap()
    bqs = nc.dram_tensor("bqs", [128, 2], dt.float32, kind="ExternalInput").ap()
    bks = nc.dram_tensor("bks", [128, 2], dt.float32, kind="ExternalInput").ap()
    bvb = nc.dram_tensor("bvb", [128, DH], dt.float32, kind="ExternalInput").ap()
    out = nc.dram_tensor("out", [S, D_MODEL], dt.float32, kind="ExternalOutput").ap()

    wq_d = WqT.rearrange("(c p) d -> c p d", p=128)
    wk_d = WkT.rearrange("(c p) d -> c p d", p=128)
    wv_d = WvT.rearrange("(c p) d -> c p d", p=128)

    with tile.TileContext(nc) as tc:
        with (
            tc.tile_pool(name="weights", bufs=1) as wpool,
            tc.tile_pool(name="acts", bufs=1) as apool,
            tc.tile_pool(name="pt", bufs=PT_BUFS) as ptpool,
            tc.tile_pool(name="sm", bufs=4) as smpool,
            tc.tile_pool(name="outsb", bufs=4) as opool,
            tc.tile_pool(name="mm", bufs=MM_BUFS, space="PSUM") as mmpool,
            tc.tile_pool(name="sc", bufs=SC_BUFS, space="PSUM") as scpool,
            tc.tile_pool(name="pv", bufs=1, space="PSUM") as pvpool,
        ):
            # ---- input loads: one big strided DMA per tensor / q-tile.
            # Each dma_start costs ~1.3us of issuing-sequencer time, so bulk
            # loads must be few and kept off the compute engines' sequencers
            # (everything goes through SP, ordered by first use).
            xT_t = [wpool.tile([128, CCH, 512], dt.float32r, name=f"xTt{t}",
                               tag=f"xTt{t}") for t in range(QT_TILES)]
            xT_r = xT.rearrange("(c p) q -> p c q", p=128)
            Wq_sb = wpool.tile([128, CCH, DH], dt.float32r)
            Wk_sb = wpool.tile([128, CCH, DH], dt.float32r)
            Wv_sb = wpool.tile([128, CCH, DH], dt.float32r)
            wq_r = WqT.rearrange("(c p) d -> p c d", p=128)
            # interleave chunk-halves so the first QK matmuls start early
            nc.sync.dma_start(Wq_sb[:, 0:4, :], wq_r[:, 0:4, :])
            nc.sync.dma_start(xT_t[0][:, 0:4, :], xT_r[:, 0:4, 0:512])
            nc.sync.dma_start(Wq_sb[:, 4:8, :], wq_r[:, 4:8, :])
            nc.sync.dma_start(xT_t[0][:, 4:8, :], xT_r[:, 4:8, 0:512])
            bqs_sb = wpool.tile([128, 2], dt.float32)
            nc.sync.dma_start(bqs_sb[:], bqs)
            nc.sync.dma_start(Wk_sb[:], WkT.rearrange("(c p) d -> p c d", p=128))
            bks_sb = wpool.tile([128, 2], dt.float32)
            nc.sync.dma_start(bks_sb[:], bks)
            nc.sync.dma_start(Wv_sb[:], WvT.rearrange("(c p) d -> p c d", p=128))
            bvb_sb = wpool.tile([128, DH], dt.float32)
            nc.sync.dma_start(bvb_sb[:], bvb)
            WoT_sb = [wpool.tile([128, D_MODEL], dt.float32r, name=f"Wo{j}",
                                 tag=f"Wo{j}") for j in range(2)]

            def emit_xt_load(t):
                # paced prefetch: issued one q-tile ahead
                nc.sync.dma_start(xT_t[t][:],
                                  xT_r[:, :, t * 512:(t + 1) * 512])

            # ---- fully streamed per q-tile: projections for tile t, then
            # attention for tile t (overlaps next tile's projections on PE),
            # then tile t's slice of the output projection.
            QT_sb = [[apool.tile([128, 512], dt.float32r, name=f"QT{j}_{t}",
                                 tag=f"QT{j}_{t}") for t in range(QT_TILES)]
                     for j in range(2)]
            KT_sb = [[apool.tile([128, 512], dt.float32r, name=f"KT{j}_{t}",
                                 tag=f"KT{j}_{t}") for t in range(QT_TILES)]
                     for j in range(2)]
            V_sb = [apool.tile([128, H_PER_CORE * VW], dt.float32r,
                               name=f"V{k}", tag=f"V{k}") for k in range(KCH)]
            OT_sb = [[apool.tile([128, 512], dt.float32r, name=f"OT{j}_{t}",
                                 tag=f"OT{j}_{t}") for t in range(QT_TILES)]
                     for j in range(2)]

            def emit_qk_proj(w, j, t):
                w_sb, o_sb, b_sb = ((Wq_sb, QT_sb, bqs_sb),
                                    (Wk_sb, KT_sb, bks_sb))[w]
                ps = mmpool.tile([128, 512], dt.float32, name="ps",
                                 tag="ps")
                for c in range(CCH):
                    nc.tensor.matmul(
                        ps[:],
                        w_sb[:, c, j * 128:(j + 1) * 128],
                        xT_t[t][:, c, :],
                        start=(c == 0), stop=(c == CCH - 1))
                nc.vector.tensor_scalar_add(
                    o_sb[j][t][:], ps[:], b_sb[:, j:j + 1])

            def emit_v_proj(k):
                ps = mmpool.tile([128, DH], dt.float32, name="ps", tag="ps")
                for c in range(CCH):
                    nc.tensor.matmul(
                        ps[:],
                        xT_t[k // 4][:, c, (k % 4) * 128:(k % 4 + 1) * 128],
                        Wv_sb[:, c, :],
                        start=(c == 0), stop=(c == CCH - 1))
                v_dst = V_sb[k][:].rearrange("p (h e) -> p h e", e=VW)[:, :, 0:D_K]
                nc.vector.tensor_tensor(
                    out=v_dst,
                    in0=ps[:].rearrange("p (h e) -> p h e", e=D_K),
                    in1=bvb_sb[:].rearrange("p (h e) -> p h e", e=D_K),
                    op=mybir.AluOpType.add)
                od = V_sb[k][:].rearrange("p (h e) -> p h e", e=VW)[:, :, D_K]
                nc.vector.tensor_scalar(
                    out=od, in0=bvb_sb[:, 0:H_PER_CORE], scalar1=0.0,
                    scalar2=1.0, op0=mybir.AluOpType.mult,
                    op1=mybir.AluOpType.add)

            # work queues drained one item per attention pair-stage: proj
            # groups for the next q-tile first (they gate the next tile's
            # attention), then output-projection groups (no dependents).
            pending_proj = []
            pending_oproj = []

            def drain_one():
                if pending_proj:
                    pending_proj.pop(0)()
                elif pending_oproj:
                    emit_oproj_group(*pending_oproj.pop(0))

            def emit_oproj_group(t, tt, on_act=False):
                o_sb = opool.tile([128, 2, 512], dt.float32, name="osb",
                                  tag="osb", bufs=2)
                for m in range(2):
                    ps = mmpool.tile([128, 512], dt.float32, name="ps",
                                     tag="ps")
                    for j in range(2):
                        nc.tensor.matmul(
                            ps[:],
                            OT_sb[j][t][:, (tt % 4) * 128:(tt % 4 + 1) * 128],
                            WoT_sb[j][:, m * 512:(m + 1) * 512],
                            start=(j == 0), stop=(j == 1))
                    if on_act:
                        # tail groups: copy on the (idle) ACT engine and DMA
                        # each half out as soon as it lands
                        nc.scalar.copy(o_sb[:, m, :], ps[:])
                        nc.sync.dma_start(
                            out[tt * 128:(tt + 1) * 128,
                                m * 512:(m + 1) * 512],
                            o_sb[:, m, :])
                    else:
                        nc.vector.tensor_copy(o_sb[:, m, :], ps[:])
                if not on_act:
                    nc.sync.dma_start(out[tt * 128:(tt + 1) * 128, :], o_sb[:])

            from functools import partial

            def queue_proj(t):
                pending_proj.append(partial(emit_qk_proj, 0, 0, t))
                pending_proj.append(partial(emit_qk_proj, 1, 0, t))
                for k in range(4 * t, 4 * t + 4):
                    pending_proj.append(partial(emit_v_proj, k))
                pending_proj.append(partial(emit_qk_proj, 0, 1, t))
                pending_proj.append(partial(emit_qk_proj, 1, 1, t))

            queue_proj(0)
            for t in range(QT_TILES):
                npair = 2 * (t + 1)
                if t + 1 < QT_TILES:
                    emit_xt_load(t + 1)
                if t == 0:
                    for j in range(2):
                        nc.sync.dma_start(WoT_sb[j][:],
                                          WoT[j * 128:(j + 1) * 128, :])
                # flush any proj groups for this tile not yet drained
                while pending_proj:
                    pending_proj.pop(0)()
                if t + 1 < QT_TILES:
                    queue_proj(t + 1)
                for j in range(2):  # head pair (heads 2j, 2j+1)
                    pvs = [pvpool.tile([VW, 512], dt.float32, name=f"pv{p}",
                                       tag=f"pv{p}", bufs=PV_BUFS) for p in range(2)]
                    # Diagonal pairs first: pair 2t covers all pv columns with
                    # its two start=True matmuls; later pairs accumulate.
                    # Sub-diagonal exp garbage is excluded by restricting the
                    # PV matmul column ranges; only the two 128x128 triangle
                    # blocks get masked (single strided affine_select).
                    # Software-pipelined by one pair-stage: PV matmuls of pair
                    # i-1 are emitted after pair i's scores+exp, so the PE
                    # never head-of-line blocks on an exp in flight.
                    def flush(mms_pp, final):
                        for p in range(2):
                            for i, (lhsT, rhs, o, st) in enumerate(mms_pp[p]):
                                nc.tensor.matmul(
                                    o, lhsT, rhs, start=st,
                                    stop=(final and i == len(mms_pp[p]) - 1),
                                    skip_group_check=True)

                    prev = None
                    for pii, pi in enumerate(range(npair)):
                        diag = pi >= 2 * t
                        first_pair = pii == 0
                        # for the second diagonal pair (r0=2) only columns
                        # q >= 128*r0 can be unmasked for either half, so the
                        # scores matmuls and exp skip the dead columns.
                        q0 = 128 * 2 * (pi - 2 * t) if diag else 0
                        pts = []
                        for p in range(2):
                            lo = p * 64
                            sc = scpool.tile([128, 2, 512], dt.float32,
                                             name="sc", tag="sc")
                            for half in range(2):
                                c = 2 * pi + half
                                nc.tensor.matmul(
                                    sc[:, half, q0:],
                                    KT_sb[j][(2 * pi) // 4][
                                        lo:lo + 64,
                                        (c % 4) * 128:(c % 4 + 1) * 128],
                                    QT_sb[j][t][lo:lo + 64, q0:],
                                    start=True, stop=True,
                                    tile_position=(lo, 0))
                            pt = ptpool.tile([128, 2, 512], dt.float32r,
                                             name="pt", tag="pt")
                            nc.scalar.activation(pt[:, :, q0:], sc[:, :, q0:],
                                                 AF.Exp)
                            pts.append(pt)
                        mms_pp = []
                        for p in range(2):
                            pt = pts[p]
                            h = 2 * j + p
                            vsl = lambda c: V_sb[c][:, h * VW:(h + 1) * VW]
                            mms = []  # (lhsT, rhs, out, start)
                            if not diag:
                                for half in range(2):
                                    mms.append((vsl(2 * pi + half),
                                                pt[:, half, :], pvs[p][:],
                                                first_pair and half == 0))
                            else:
                                r0 = 2 * (pi - 2 * t)
                                # zero below-diagonal of each 128x128
                                # triangle block
                                for half in range(2):
                                    r = r0 + half
                                    tri = pt[:, half, 128 * r:128 * (r + 1)]
                                    nc.gpsimd.affine_select(
                                        out=tri, in_=tri,
                                        compare_op=mybir.AluOpType.is_ge,
                                        fill=0.0, base=0,
                                        pattern=[[1, 128]],
                                        channel_multiplier=-1)
                                for half in range(2):
                                    # one matmul over [128r, 512): triangle
                                    # (masked) + fully-valid columns
                                    r = r0 + half
                                    mms.append(
                                        (vsl(2 * pi + half),
                                         pt[:, half, 128 * r:],
                                         pvs[p][:, 128 * r:],
                                         first_pair and half == 0))
                            mms_pp.append(mms)
                        if prev is not None:
                            drain_one()
                            flush(prev, False)
                        prev = mms_pp
                    flush(prev, True)
                    for p in range(2):
                        rc = smpool.tile([1, 512], dt.float32, name="rc",
                                         tag="rc", bufs=2)
                        nc.vector.reciprocal(rc[:], pvs[p][D_K:VW, :])
                        bc = smpool.tile([64, 512], dt.float32, name="bc",
                                         tag="bc", bufs=2)
                        nc.gpsimd.partition_broadcast(bc[:], rc[:])
                        nc.vector.tensor_tensor(
                            out=OT_sb[j][t][p * 64:(p + 1) * 64, :],
                            in0=pvs[p][0:D_K, :], in1=bc[:],
                            op=mybir.AluOpType.mult)


                # queue this q-tile's output projection; drained one group
                # per attention pair-stage of later tiles (fills PE slack
                # while ACT is the stage bottleneck)
                for tt in range(4 * t, 4 * t + 4):
                    pending_oproj.append((t, tt))
            for args in pending_oproj:
                emit_oproj_group(*args, on_act=True)
    nc.compile()
    return nc


def _in_maps(x, Wq, bq, Wk, bk, Wv, bv, Wo, bo):
    maps = []
    for core in range(N_CORES):
        b = core // 4
        h0 = (core % 4) * H_PER_CORE
        hs = slice(h0 * D_K, (h0 + H_PER_CORE) * D_K)
        m = {
            "xT": np.ascontiguousarray(x[b].T),
            "WqT": np.ascontiguousarray(Wq[hs, :].T) * 0.125,
            "WkT": np.ascontiguousarray(Wk[hs, :].T),
            "WvT": np.ascontiguousarray(Wv[hs, :].T),
            "WoT": np.ascontiguousarray(Wo[:, hs].T),
            "bqs": np.ascontiguousarray((bq[hs] * 0.125).reshape(2, 128).T),
            "bks": np.ascontiguousarray(bk[hs].reshape(2, 128).T),
            "bvb": np.ascontiguousarray(
                np.broadcast_to(bv[hs], (128, DH))),
        }
        maps.append({k: np.ascontiguousarray(v, dtype=np.float32)
                     for k, v in m.items()})
    return maps


def kernel(x, Wq, bq, Wk, bk, Wv, bv, Wo, bo, _trace=False):
    if "nc" not in _CACHE:
        _CACHE["nc"] = _build()
    nc = _CACHE["nc"]
    in_maps = _in_maps(np.asarray(x, dtype=np.float32),
                       np.asarray(Wq, dtype=np.float32),
                       np.asarray(bq, dtype=np.float32),
                       np.asarray(Wk, dtype=np.float32),
                       np.asarray(bk, dtype=np.float32),
                       np.asarray(Wv, dtype=np.float32),
                       np.asarray(bv, dtype=np.float32),
                       np.asarray(Wo, dtype=np.float32),
                       np.asarray(bo, dtype=np.float32))
    res = run_bass_kernel_spmd(nc, in_maps, core_ids=list(range(N_CORES)),
                               trace=_trace)
    bo = np.asarray(bo, dtype=np.float32)
    out = np.zeros((B, S, D_MODEL), dtype=np.float32)
    for b in range(B):
        acc = res.results[b * 4]["out"].astype(np.float64)
        for core in range(b * 4 + 1, b * 4 + 4):
            acc = acc + res.results[core]["out"]
        out[b] = (acc + bo).astype(np.float32)
    if _trace:
        return out, res
    return out
